# revision 41
# baseline (speedup 1.0000x reference)
"""Additive-attention pooling kernel for 8 TRN2 NeuronCores.

Problem (per full input):
    u = tanh(value @ W1^T + query @ W2^T + b)          # [B, S, H]
    scores = u @ w, masked to s < lens[b], softmax over s
    out = sum_s softmax(scores)[b, s] * value[b, s, :]  # [B, DV]

Sharding: data-parallel over the batch dim (4 batches per core); the small
parameters (W1, W2, b, w) are replicated.

Per-core pipeline (matmuls in bf16, f32 PSUM accumulation):
  1. SWDGE DMAs load value in 1MB chunks, casting f32->bf16 in the DMA
     datapath, into nat[p, t, v] = value[128t + p, v].
  2. TensorE identity-transposes (transpose mode, bf16 PSUM, 8 tiles per
     bank) produce valueT[v, s]; VectorE evacuates at 2x.
  3. u-matmul: W1T chunks stationary, valueT moving; ScalarE tanh with
     per-partition bias (c = query@W2^T + b) writes uT bf16 to SBUF.
  4. Scores: M=32 matmuls (w replicated across 32 columns) put batch b in
     partition block [32b, 32b+32) -- compute engines cannot address
     strided partition rows, so the replicated block keeps everything
     legal.  The whole scores->exp->e-transpose->mask chain runs at
     512-column eighths inside the phase-A PSUM scope so the scheduler
     hoists it into pipeline gaps as batches complete.
  5. e redistribution without any DRAM bounce: PE-transpose each 128-col
     chunk of e (reusing the score PSUM slots); transposed column 32b
     holds e_b[128t + i] over partitions i -- exactly the pooling
     stationary vector. VectorE evacuates the 4 needed columns per chunk;
     an iota-vs-lens compare masks and replicates e into the M=32
     stationary, emitting per-partition sums via accum_out.
  6. Pooling: M=32 matmuls, each batch accumulating in its own PSUM bank;
     sum(e) reaches column DV via a reduce + one N=1 matmul against a
     1/32-filled stationary; reciprocal scale finishes the softmax.
  All small parameters ship pre-packed in one [128, 1036] image so a
  single DMA replaces eight small ones on the Sync queue.
"""

import numpy as np

import concourse.bass as bass
import concourse.bacc as bacc
import concourse.tile as tile
from concourse import mybir
from concourse.bass_utils import run_bass_kernel_spmd


B, S, DV, DQ, H = 32, 4096, 256, 256, 256
NCORES = 8
BL = B // NCORES  # batches per core

ST = S // 128     # 32 s-tiles of 128
NCH = 4           # load chunks per batch
CT = ST // NCH    # tiles per chunk
PW = 1036         # packed params width: w1t 512 | w2t 512 | w 2 | b 2 | qT 8
F32 = mybir.dt.float32
BF16 = mybir.dt.bfloat16
I32 = mybir.dt.int32


def build_nc():
    nc = bacc.Bacc("TRN2", target_bir_lowering=False)

    value_ext = nc.declare_dram_parameter("value", [BL, S, DV], F32, isOutput=False)
    lens_ext = nc.declare_dram_parameter("lens", [BL], I32, isOutput=False)
    # all small f32 parameters pre-packed host-side into one SBUF-layout
    # image: [w1t | w2t | w | b | qT] -> a single DMA instead of eight
    params_ext = nc.declare_dram_parameter(
        "params", [128, PW], F32, isOutput=False
    )
    out_ext = nc.declare_dram_parameter("out", [BL, DV], F32, isOutput=True)

    Tanh = mybir.ActivationFunctionType.Tanh
    Exp = mybir.ActivationFunctionType.Exp
    Alu = mybir.AluOpType

    with tile.TileContext(nc) as tc:
        with (
            tc.tile_pool(name="singles", bufs=1) as singles,
            tc.tile_pool(name="nat", bufs=BL) as nat_pool,
            tc.tile_pool(name="vt", bufs=20) as vt_pool,
            tc.tile_pool(name="ut", bufs=2 * BL) as ut_pool,
        ):
            # ---- iotas first (cheap; keeps the load-DMA queue behind them short) --
            io_col = singles.tile([128, 128], I32, tag="io_col")
            io_row = singles.tile([128, 128], I32, tag="io_row")
            nc.gpsimd.iota(io_col, [[1, 128]], channel_multiplier=0)
            nc.gpsimd.iota(io_row, [[0, 128]], channel_multiplier=1)
            identity = singles.tile([128, 128], BF16, tag="identity")
            nc.vector.tensor_tensor(identity, io_row, io_col, Alu.is_equal)

            # s-index iota for the length mask: val[p, t] = 128t + p
            iota_s = singles.tile([128, ST], F32, tag="iota_s")
            nc.gpsimd.iota(
                iota_s, [[128, ST]], channel_multiplier=1,
                allow_small_or_imprecise_dtypes=True,
            )

            # ---- value loads first: SWDGE cast-DMAs (f32->bf16) --------
            nat = []
            for b in range(BL):
                natb = nat_pool.tile([128, ST, DV], BF16, tag="nat")
                nat.append(natb)
            for b in range(BL):
                for ch in range(4):
                    src = value_ext[b, ch * 1024:(ch + 1) * 1024, :]
                    nc.gpsimd.dma_start(
                        out=nat[b][:, ch * 8:(ch + 1) * 8, :],
                        in_=src.rearrange("(t p) v -> p t v", p=128),
                    )

            params_sb = singles.tile([128, PW], F32, tag="params_sb")
            nc.sync.dma_start(out=params_sb, in_=params_ext[:, :])
            w1t_f = params_sb[:, 0:512].rearrange("p (c h) -> p c h", c=2)
            w2t_f = params_sb[:, 512:1024].rearrange("p (c h) -> p c h", c=2)
            w_f = params_sb[:, 1024:1026]
            b_sb = params_sb[:, 1026:1028]
            qT = params_sb[:, 1028:1036].rearrange("p (c b) -> p c b", c=2)

            lens_i = singles.tile([128, BL], I32, tag="lens_i")
            nc.sync.dma_start(
                out=lens_i,
                in_=bass.AP(tensor=lens_ext, offset=0, ap=[[0, 128], [1, BL]]),
            )
            lens_f = singles.tile([128, BL], F32, tag="lens_f")
            nc.vector.tensor_copy(lens_f, lens_i)

            w1t_bf = singles.tile([128, 2, H], BF16, tag="w1t_bf")
            nc.vector.tensor_copy(w1t_bf, w1t_f)

            zero32 = singles.tile([128, 32], BF16, tag="zero32")
            nc.vector.memset(zero32, 0.0)
            w_rep = singles.tile([128, 2, 32], BF16, tag="w_rep")
            for hh in range(2):
                nc.vector.tensor_scalar(
                    w_rep[:, hh, :], zero32, w_f[:, hh:hh + 1], None, Alu.add
                )

            # 1/32-filled stationary for the sum(e) matmul
            ones_rep = singles.tile([128, 32], BF16, tag="ones_rep")
            nc.vector.memset(ones_rep, 1.0 / 32.0)

            # c[b, h] = query[b] @ W2^T + b   ->  cT [128h, hh, b] f32
            cT = singles.tile([128, 2, BL], F32, tag="cT")
            with tc.tile_pool(name="ct_ps", bufs=2, space="PSUM") as ct_pool:
                for hh in range(2):
                    ct_ps = ct_pool.tile([128, BL], F32, tag="ct")
                    for c in range(2):
                        nc.tensor.matmul(
                            ct_ps,
                            w2t_f[:, c, hh * 128:(hh + 1) * 128],
                            qT[:, c, :],
                            start=(c == 0),
                            stop=(c == 1),
                        )
                    nc.vector.tensor_scalar(
                        cT[:, hh, :], ct_ps, b_sb[:, hh:hh + 1], None, Alu.add
                    )

            # ---- phase A: transpose + u-matmul + tanh, per batch -------
            ut = [None] * (2 * BL)
            with (
                tc.tile_pool(name="tp_ps", bufs=2, space="PSUM") as tp_pool,
                tc.tile_pool(name="up_ps", bufs=2, space="PSUM") as up_pool,
            ):
                for b in range(BL):
                    vts = {}
                    for g in range(4):
                        for vh in range(2):
                            vt = vt_pool.tile([128, 1024], BF16, tag="vt")
                            vts[(vh, g)] = vt
                            tp = tp_pool.tile([128, 1024], BF16, tag="tp")
                            for k in range(8):
                                t = g * 8 + k
                                nc.tensor.matmul(
                                    tp[:, k * 128:(k + 1) * 128],
                                    nat[b][:, t, vh * 128:(vh + 1) * 128],
                                    identity,
                                    is_transpose=True,
                                    start=(k == 0),
                                    stop=(k == 7),
                                )
                            nc.vector.tensor_copy(vt, tp)
                    for g in range(4):
                        for hh in range(2):
                            if g == 0:
                                utb = ut_pool.tile([128, S], BF16, tag="ut")
                                ut[2 * b + hh] = utb
                            utb = ut[2 * b + hh]
                            up = up_pool.tile([128, 1024], F32, tag="up")
                            for sc in range(2):
                                lo = sc * 512
                                for vh in range(2):
                                    nc.tensor.matmul(
                                        up[:, lo:lo + 512],
                                        w1t_bf[:, vh, hh * 128:(hh + 1) * 128],
                                        vts[(vh, g)][:, lo:lo + 512],
                                        start=(vh == 0),
                                        stop=(vh == 1),
                                    )
                            nc.scalar.activation(
                                utb[:, g * 1024:(g + 1) * 1024],
                                up,
                                Tanh,
                                bias=cT[:, hh, b:b + 1],
                                scale=1.0,
                            )

                # scores/exp/e-transpose/mask run at 512-column eighths in
                # this PSUM scope so the scheduler can hoist them into
                # phase-A gaps as batches complete; the e-transposes reuse
                # the score bank slots (sc freed by exp, et is a new slot)
                e_sb = singles.tile([128, S], BF16, tag="e_sb")
                e_resh = singles.tile([128, ST, BL], BF16, tag="e_resh")
                e_rep = singles.tile([128, BL, ST, 32], BF16, tag="e_rep")
                psums = singles.tile([128, BL, 8], F32, tag="psums")
                with tc.tile_pool(name="se_ps", bufs=2, space="PSUM") as se_pool:
                    for e8 in range(8):
                        soff = e8 * 512
                        toff = e8 * 4
                        sc_ps = se_pool.tile([128, 512], F32, tag="se")
                        for b in range(BL):
                            for hh in range(2):
                                nc.tensor.matmul(
                                    sc_ps[32 * b:32 * b + 32, :],
                                    w_rep[:, hh, :],
                                    ut[2 * b + hh][:, soff:soff + 512],
                                    start=(hh == 0),
                                    stop=(hh == 1),
                                    tile_position=(0, 32 * b),
                                )
                        nc.scalar.activation(
                            e_sb[:, soff:soff + 512], sc_ps, Exp
                        )
                        et = se_pool.tile([128, 512], F32, tag="se")
                        etb = et.bitcast(BF16)[:, 0:512]
                        for j in range(4):
                            nc.tensor.matmul(
                                etb[:, j * 128:(j + 1) * 128],
                                e_sb[:, soff + j * 128:soff + (j + 1) * 128],
                                identity,
                                is_transpose=True,
                                start=(j == 0),
                                stop=(j == 3),
                            )
                        ev = etb.rearrange("p (t c) -> p t c", c=128)
                        nc.vector.tensor_copy(
                            e_resh[:, toff:toff + 4, :],
                            ev.rearrange("p t (bb x) -> p t bb x", x=32)[:, :, :, 0],
                        )
                        for b in range(BL):
                            er = e_resh[:, toff:toff + 4, b]
                            er_b = bass.AP(tensor=er.tensor, offset=er.offset,
                                           ap=[*er.ap, [0, 32]])
                            io = iota_s[:, toff:toff + 4]
                            io_b = bass.AP(tensor=io.tensor, offset=io.offset,
                                           ap=[*io.ap, [0, 32]])
                            nc.vector.scalar_tensor_tensor(
                                e_rep[:, b, toff:toff + 4],
                                io_b,
                                lens_f[:, b:b + 1],
                                er_b,
                                Alu.is_lt,
                                Alu.mult,
                                accum_out=psums[:, b, e8:e8 + 1],
                            )

            # ---- phase C: pooling + normalization ----------------------
            psums_r = singles.tile([128, BL], F32, tag="psums_r")
            psums_bf = singles.tile([128, BL], BF16, tag="psums_bf")
            out_sb = singles.tile([128, DV], F32, tag="out_sb")
            sums_r = singles.tile([128, 1], F32, tag="sums_r")

            with tc.tile_pool(name="po_ps", bufs=1, space="PSUM") as po_pool:
                po_ps = po_pool.tile([128, BL, 512], F32, tag="po")
                for t in range(ST):
                    for b in range(BL):
                        nc.tensor.matmul(
                            po_ps[32 * b:32 * b + 32, b, 0:DV],
                            e_rep[:, b, t, :],
                            nat[b][:, t, :],
                            start=(t == 0),
                            stop=(t == ST - 1),
                            tile_position=(0, 32 * b),
                        )

                # sum(e): per-partition sums -> reduce over eighths -> bf16
                # -> one N=1 matmul per batch into po column DV
                nc.vector.tensor_reduce(
                    psums_r, psums, op=Alu.add, axis=mybir.AxisListType.X
                )
                nc.vector.tensor_copy(psums_bf, psums_r)
                for b in range(BL):
                    nc.tensor.matmul(
                        po_ps[32 * b:32 * b + 32, b, DV:DV + 1],
                        ones_rep,
                        psums_bf[:, b:b + 1],
                        start=True,
                        stop=True,
                        tile_position=(0, 32 * b),
                    )
                for b in range(BL):
                    rows = slice(32 * b, 32 * b + 32)
                    nc.vector.reciprocal(
                        sums_r[rows], po_ps[rows, b, DV:DV + 1]
                    )
                    nc.vector.tensor_scalar(
                        out_sb[rows], po_ps[rows, b, 0:DV], sums_r[rows],
                        None, Alu.mult,
                    )
                ob_rows = out_sb.rearrange("(a b) s -> a b s", b=32)[:, 0, :]
                nc.sync.dma_start(out=out_ext[:, :], in_=ob_rows)

    nc.compile()
    return nc


_NC_CACHE = None


def _get_nc():
    global _NC_CACHE
    if _NC_CACHE is None:
        _NC_CACHE = build_nc()
    return _NC_CACHE


def make_in_maps(value, query, lens, W1, W2, b, w):
    value = np.ascontiguousarray(np.asarray(value, dtype=np.float32))
    query = np.asarray(query, dtype=np.float32)
    lens = np.ascontiguousarray(np.asarray(lens, dtype=np.int32))
    w1t = np.asarray(W1, dtype=np.float32).T
    w2t = np.asarray(W2, dtype=np.float32).T
    bvec = np.asarray(b, dtype=np.float32).reshape(H)
    wvec = np.asarray(w, dtype=np.float32).reshape(H)

    def pack(core):
        sl = slice(core * BL, (core + 1) * BL)
        P = np.zeros((128, PW), np.float32)
        P[:, 0:512] = w1t.reshape(2, 128, H).transpose(1, 0, 2).reshape(128, 512)
        P[:, 512:1024] = w2t.reshape(2, 128, H).transpose(1, 0, 2).reshape(128, 512)
        P[:, 1024:1026] = wvec.reshape(2, 128).T
        P[:, 1026:1028] = bvec.reshape(2, 128).T
        P[:, 1028:1036] = (
            query[sl].T.reshape(2, 128, BL).transpose(1, 0, 2).reshape(128, 2 * BL)
        )
        return np.ascontiguousarray(P)

    in_maps = []
    for i in range(NCORES):
        sl = slice(i * BL, (i + 1) * BL)
        in_maps.append({
            "value": value[sl],
            "lens": lens[sl],
            "params": pack(i),
        })
    return in_maps


def _axon_reset():
    # clear a wedged exec unit left over from a previous crashed run
    try:
        import ctypes
        import jax
        jax.devices()
        lib = ctypes.CDLL("/opt/axon/libaxon_pjrt.so")
        lib.axon_reset.restype = ctypes.c_int64
        lib.axon_reset()
    except Exception:
        pass


def kernel(value, query, lens, W1, W2, b, w):
    nc = _get_nc()
    in_maps = make_in_maps(value, query, lens, W1, W2, b, w)
    try:
        res = run_bass_kernel_spmd(nc, in_maps, core_ids=list(range(NCORES)))
    except Exception:
        _axon_reset()
        res = run_bass_kernel_spmd(nc, in_maps, core_ids=list(range(NCORES)))
    out = np.concatenate(
        [np.asarray(res.results[i]["out"]) for i in range(NCORES)], axis=0
    )
    return out.astype(np.float32)


# revision 42
# speedup vs baseline: 1.0179x; 1.0179x over previous
"""Additive-attention pooling kernel for 8 TRN2 NeuronCores.

Problem (per full input):
    u = tanh(value @ W1^T + query @ W2^T + b)          # [B, S, H]
    scores = u @ w, masked to s < lens[b], softmax over s
    out = sum_s softmax(scores)[b, s] * value[b, s, :]  # [B, DV]

Sharding: data-parallel over the batch dim (4 batches per core); the small
parameters (W1, W2, b, w) are replicated.

Per-core pipeline (matmuls in bf16, f32 PSUM accumulation):
  1. SWDGE DMAs load value in 1MB chunks, casting f32->bf16 in the DMA
     datapath, into nat[p, t, v] = value[128t + p, v].
  2. TensorE identity-transposes (transpose mode, bf16 PSUM, 8 tiles per
     bank) produce valueT[v, s]; VectorE evacuates at 2x.
  3. u-matmul: W1T chunks stationary, valueT moving; ScalarE tanh with
     per-partition bias (c = query@W2^T + b) writes uT bf16 to SBUF.
  4. Scores: M=32 matmuls (w replicated across 32 columns) put batch b in
     partition block [32b, 32b+32) -- compute engines cannot address
     strided partition rows, so the replicated block keeps everything
     legal.  The whole scores->exp->e-transpose->mask chain runs at
     512-column eighths inside the phase-A PSUM scope so the scheduler
     hoists it into pipeline gaps as batches complete.
  5. e redistribution without any DRAM bounce: PE-transpose each 128-col
     chunk of e (reusing the score PSUM slots); transposed column 32b
     holds e_b[128t + i] over partitions i -- exactly the pooling
     stationary vector. VectorE evacuates the 4 needed columns per chunk;
     an iota-vs-lens compare masks and replicates e into the M=32
     stationary, emitting per-partition sums via accum_out.
  6. Pooling: M=32 matmuls, each batch accumulating in its own PSUM bank;
     sum(e) reaches column DV via a reduce + one N=1 matmul against a
     1/32-filled stationary; reciprocal scale finishes the softmax.
  All small parameters ship pre-packed in one [128, 1036] image so a
  single DMA replaces eight small ones on the Sync queue.
"""

import numpy as np

import concourse.bass as bass
import concourse.bacc as bacc
import concourse.tile as tile
from concourse import mybir
from concourse.bass_utils import run_bass_kernel_spmd


B, S, DV, DQ, H = 32, 4096, 256, 256, 256
NCORES = 8
BL = B // NCORES  # batches per core

ST = S // 128     # 32 s-tiles of 128
NCH = 4           # load chunks per batch
CT = ST // NCH    # tiles per chunk
PW = 1036         # packed params width: w1t 512 | w2t 512 | w 2 | b 2 | qT 8
F32 = mybir.dt.float32
BF16 = mybir.dt.bfloat16
I32 = mybir.dt.int32


def build_nc():
    nc = bacc.Bacc("TRN2", target_bir_lowering=False)

    value_ext = nc.declare_dram_parameter("value", [BL, S, DV], F32, isOutput=False)
    lens_ext = nc.declare_dram_parameter("lens", [BL], I32, isOutput=False)
    # all small f32 parameters pre-packed host-side into one SBUF-layout
    # image: [w1t | w2t | w | b | qT] -> a single DMA instead of eight
    params_ext = nc.declare_dram_parameter(
        "params", [128, PW], F32, isOutput=False
    )
    out_ext = nc.declare_dram_parameter("out", [BL, DV], F32, isOutput=True)

    Tanh = mybir.ActivationFunctionType.Tanh
    Exp = mybir.ActivationFunctionType.Exp
    Alu = mybir.AluOpType

    with tile.TileContext(nc) as tc:
        with (
            tc.tile_pool(name="singles", bufs=1) as singles,
            tc.tile_pool(name="nat", bufs=BL) as nat_pool,
            tc.tile_pool(name="vt", bufs=16) as vt_pool,
            tc.tile_pool(name="ut", bufs=2 * BL) as ut_pool,
        ):
            # ---- iotas first (cheap; keeps the load-DMA queue behind them short) --
            io_col = singles.tile([128, 128], I32, tag="io_col")
            io_row = singles.tile([128, 128], I32, tag="io_row")
            nc.gpsimd.iota(io_col, [[1, 128]], channel_multiplier=0)
            nc.gpsimd.iota(io_row, [[0, 128]], channel_multiplier=1)
            identity = singles.tile([128, 128], BF16, tag="identity")
            nc.vector.tensor_tensor(identity, io_row, io_col, Alu.is_equal)

            # s-index iota for the length mask: val[p, t] = 128t + p
            iota_s = singles.tile([128, ST], F32, tag="iota_s")
            nc.gpsimd.iota(
                iota_s, [[128, ST]], channel_multiplier=1,
                allow_small_or_imprecise_dtypes=True,
            )

            # ---- value loads first: SWDGE cast-DMAs (f32->bf16) --------
            nat = []
            for b in range(BL):
                natb = nat_pool.tile([128, ST, DV], BF16, tag="nat")
                nat.append(natb)
            for b in range(BL):
                for ch in range(4):
                    src = value_ext[b, ch * 1024:(ch + 1) * 1024, :]
                    nc.gpsimd.dma_start(
                        out=nat[b][:, ch * 8:(ch + 1) * 8, :],
                        in_=src.rearrange("(t p) v -> p t v", p=128),
                    )

            params_sb = singles.tile([128, PW], F32, tag="params_sb")
            nc.sync.dma_start(out=params_sb, in_=params_ext[:, :])
            w1t_f = params_sb[:, 0:512].rearrange("p (c h) -> p c h", c=2)
            w2t_f = params_sb[:, 512:1024].rearrange("p (c h) -> p c h", c=2)
            w_f = params_sb[:, 1024:1026]
            b_sb = params_sb[:, 1026:1028]
            qT = params_sb[:, 1028:1036].rearrange("p (c b) -> p c b", c=2)

            lens_i = singles.tile([128, BL], I32, tag="lens_i")
            nc.sync.dma_start(
                out=lens_i,
                in_=bass.AP(tensor=lens_ext, offset=0, ap=[[0, 128], [1, BL]]),
            )
            lens_f = singles.tile([128, BL], F32, tag="lens_f")
            nc.vector.tensor_copy(lens_f, lens_i)

            w1t_bf = singles.tile([128, 2, H], BF16, tag="w1t_bf")
            nc.vector.tensor_copy(w1t_bf, w1t_f)

            zero32 = singles.tile([128, 32], BF16, tag="zero32")
            nc.vector.memset(zero32, 0.0)
            w_rep = singles.tile([128, 2, 32], BF16, tag="w_rep")
            for hh in range(2):
                nc.vector.tensor_scalar(
                    w_rep[:, hh, :], zero32, w_f[:, hh:hh + 1], None, Alu.add
                )

            # 1/32-filled stationary for the sum(e) matmul
            ones_rep = singles.tile([128, 32], BF16, tag="ones_rep")
            nc.vector.memset(ones_rep, 1.0 / 32.0)

            # c[b, h] = query[b] @ W2^T + b   ->  cT [128h, hh, b] f32
            cT = singles.tile([128, 2, BL], F32, tag="cT")
            with tc.tile_pool(name="ct_ps", bufs=2, space="PSUM") as ct_pool:
                for hh in range(2):
                    ct_ps = ct_pool.tile([128, BL], F32, tag="ct")
                    for c in range(2):
                        nc.tensor.matmul(
                            ct_ps,
                            w2t_f[:, c, hh * 128:(hh + 1) * 128],
                            qT[:, c, :],
                            start=(c == 0),
                            stop=(c == 1),
                        )
                    nc.vector.tensor_scalar(
                        cT[:, hh, :], ct_ps, b_sb[:, hh:hh + 1], None, Alu.add
                    )

            # ---- phase A: transpose + u-matmul + tanh, per batch -------
            ut = [None] * (2 * BL)
            with (
                tc.tile_pool(name="tp_ps", bufs=2, space="PSUM") as tp_pool,
                tc.tile_pool(name="up_ps", bufs=2, space="PSUM") as up_pool,
            ):
                for b in range(BL):
                    vts = {}
                    for g in range(4):
                        for vh in range(2):
                            vt = vt_pool.tile([128, 1024], BF16, tag="vt")
                            vts[(vh, g)] = vt
                            tp = tp_pool.tile([128, 1024], BF16, tag="tp")
                            for k in range(8):
                                t = g * 8 + k
                                nc.tensor.matmul(
                                    tp[:, k * 128:(k + 1) * 128],
                                    nat[b][:, t, vh * 128:(vh + 1) * 128],
                                    identity,
                                    is_transpose=True,
                                    start=(k == 0),
                                    stop=(k == 7),
                                )
                            nc.vector.tensor_copy(vt, tp)
                    for g in range(4):
                        for hh in range(2):
                            if g == 0:
                                utb = ut_pool.tile([128, S], BF16, tag="ut")
                                ut[2 * b + hh] = utb
                            utb = ut[2 * b + hh]
                            up = up_pool.tile([128, 1024], F32, tag="up")
                            for sc in range(2):
                                lo = sc * 512
                                for vh in range(2):
                                    nc.tensor.matmul(
                                        up[:, lo:lo + 512],
                                        w1t_bf[:, vh, hh * 128:(hh + 1) * 128],
                                        vts[(vh, g)][:, lo:lo + 512],
                                        start=(vh == 0),
                                        stop=(vh == 1),
                                    )
                            nc.scalar.activation(
                                utb[:, g * 1024:(g + 1) * 1024],
                                up,
                                Tanh,
                                bias=cT[:, hh, b:b + 1],
                                scale=1.0,
                            )

                # scores/exp/e-transpose/mask run at 512-column eighths in
                # this PSUM scope so the scheduler can hoist them into
                # phase-A gaps as batches complete; the e-transposes reuse
                # the score bank slots (sc freed by exp, et is a new slot)
                e_sb = singles.tile([128, S], BF16, tag="e_sb")
                e_resh = singles.tile([128, ST, BL], BF16, tag="e_resh")
                e_rep = singles.tile([128, BL, ST, 32], BF16, tag="e_rep")
                psums = singles.tile([128, BL, 8], F32, tag="psums")
                with tc.tile_pool(name="se_ps", bufs=2, space="PSUM") as se_pool:
                    for e8 in range(8):
                        soff = e8 * 512
                        toff = e8 * 4
                        sc_ps = se_pool.tile([128, 512], F32, tag="se")
                        for b in range(BL):
                            for hh in range(2):
                                nc.tensor.matmul(
                                    sc_ps[32 * b:32 * b + 32, :],
                                    w_rep[:, hh, :],
                                    ut[2 * b + hh][:, soff:soff + 512],
                                    start=(hh == 0),
                                    stop=(hh == 1),
                                    tile_position=(0, 32 * b),
                                )
                        nc.scalar.activation(
                            e_sb[:, soff:soff + 512], sc_ps, Exp
                        )
                        et = se_pool.tile([128, 512], F32, tag="se")
                        etb = et.bitcast(BF16)[:, 0:512]
                        for j in range(4):
                            nc.tensor.matmul(
                                etb[:, j * 128:(j + 1) * 128],
                                e_sb[:, soff + j * 128:soff + (j + 1) * 128],
                                identity,
                                is_transpose=True,
                                start=(j == 0),
                                stop=(j == 3),
                            )
                        ev = etb.rearrange("p (t c) -> p t c", c=128)
                        nc.vector.tensor_copy(
                            e_resh[:, toff:toff + 4, :],
                            ev.rearrange("p t (bb x) -> p t bb x", x=32)[:, :, :, 0],
                        )
                        for b in range(BL):
                            er = e_resh[:, toff:toff + 4, b]
                            er_b = bass.AP(tensor=er.tensor, offset=er.offset,
                                           ap=[*er.ap, [0, 32]])
                            io = iota_s[:, toff:toff + 4]
                            io_b = bass.AP(tensor=io.tensor, offset=io.offset,
                                           ap=[*io.ap, [0, 32]])
                            nc.vector.scalar_tensor_tensor(
                                e_rep[:, b, toff:toff + 4],
                                io_b,
                                lens_f[:, b:b + 1],
                                er_b,
                                Alu.is_lt,
                                Alu.mult,
                                accum_out=psums[:, b, e8:e8 + 1],
                            )

            # ---- phase C: pooling + normalization ----------------------
            psums_r = singles.tile([128, BL], F32, tag="psums_r")
            psums_bf = singles.tile([128, BL], BF16, tag="psums_bf")
            out_sb = singles.tile([128, DV], F32, tag="out_sb")
            sums_r = singles.tile([128, 1], F32, tag="sums_r")

            with tc.tile_pool(name="po_ps", bufs=1, space="PSUM") as po_pool:
                po_ps = po_pool.tile([128, BL, 512], F32, tag="po")
                for t in range(ST):
                    for b in range(BL):
                        nc.tensor.matmul(
                            po_ps[32 * b:32 * b + 32, b, 0:DV],
                            e_rep[:, b, t, :],
                            nat[b][:, t, :],
                            start=(t == 0),
                            stop=(t == ST - 1),
                            tile_position=(0, 32 * b),
                        )

                # sum(e): per-partition sums -> reduce over eighths -> bf16
                # -> one N=1 matmul per batch into po column DV
                nc.vector.tensor_reduce(
                    psums_r, psums, op=Alu.add, axis=mybir.AxisListType.X
                )
                nc.vector.tensor_copy(psums_bf, psums_r)
                for b in range(BL):
                    nc.tensor.matmul(
                        po_ps[32 * b:32 * b + 32, b, DV:DV + 1],
                        ones_rep,
                        psums_bf[:, b:b + 1],
                        start=True,
                        stop=True,
                        tile_position=(0, 32 * b),
                    )
                for b in range(BL):
                    rows = slice(32 * b, 32 * b + 32)
                    nc.vector.reciprocal(
                        sums_r[rows], po_ps[rows, b, DV:DV + 1]
                    )
                    nc.vector.tensor_scalar(
                        out_sb[rows], po_ps[rows, b, 0:DV], sums_r[rows],
                        None, Alu.mult,
                    )
                ob_rows = out_sb.rearrange("(a b) s -> a b s", b=32)[:, 0, :]
                nc.sync.dma_start(out=out_ext[:, :], in_=ob_rows)

    nc.compile()
    return nc


_NC_CACHE = None


def _get_nc():
    global _NC_CACHE
    if _NC_CACHE is None:
        _NC_CACHE = build_nc()
    return _NC_CACHE


def make_in_maps(value, query, lens, W1, W2, b, w):
    value = np.ascontiguousarray(np.asarray(value, dtype=np.float32))
    query = np.asarray(query, dtype=np.float32)
    lens = np.ascontiguousarray(np.asarray(lens, dtype=np.int32))
    w1t = np.asarray(W1, dtype=np.float32).T
    w2t = np.asarray(W2, dtype=np.float32).T
    bvec = np.asarray(b, dtype=np.float32).reshape(H)
    wvec = np.asarray(w, dtype=np.float32).reshape(H)

    def pack(core):
        sl = slice(core * BL, (core + 1) * BL)
        P = np.zeros((128, PW), np.float32)
        P[:, 0:512] = w1t.reshape(2, 128, H).transpose(1, 0, 2).reshape(128, 512)
        P[:, 512:1024] = w2t.reshape(2, 128, H).transpose(1, 0, 2).reshape(128, 512)
        P[:, 1024:1026] = wvec.reshape(2, 128).T
        P[:, 1026:1028] = bvec.reshape(2, 128).T
        P[:, 1028:1036] = (
            query[sl].T.reshape(2, 128, BL).transpose(1, 0, 2).reshape(128, 2 * BL)
        )
        return np.ascontiguousarray(P)

    in_maps = []
    for i in range(NCORES):
        sl = slice(i * BL, (i + 1) * BL)
        in_maps.append({
            "value": value[sl],
            "lens": lens[sl],
            "params": pack(i),
        })
    return in_maps


def _axon_reset():
    # clear a wedged exec unit left over from a previous crashed run
    try:
        import ctypes
        import jax
        jax.devices()
        lib = ctypes.CDLL("/opt/axon/libaxon_pjrt.so")
        lib.axon_reset.restype = ctypes.c_int64
        lib.axon_reset()
    except Exception:
        pass


def kernel(value, query, lens, W1, W2, b, w):
    nc = _get_nc()
    in_maps = make_in_maps(value, query, lens, W1, W2, b, w)
    try:
        res = run_bass_kernel_spmd(nc, in_maps, core_ids=list(range(NCORES)))
    except Exception:
        _axon_reset()
        res = run_bass_kernel_spmd(nc, in_maps, core_ids=list(range(NCORES)))
    out = np.concatenate(
        [np.asarray(res.results[i]["out"]) for i in range(NCORES)], axis=0
    )
    return out.astype(np.float32)
